# revision 4
# baseline (speedup 1.0000x reference)
"""Distributed multi-head attention kernel for 8 TRN2 NeuronCores.

Problem: B=2, N=2048, C=1024, H=16 heads, D=64.
  out = softmax((q@Wq)(k@Wk)^T / sqrt(D)) @ (v@Wv) @ Wo   (per head, biases are zero)

Sharding: sequence-parallel within batch (2 batch groups x 4 cores).
Core c owns batch b=c//4, query rows R=[512r, 512r+512), r=c%4.

Per-core dataflow (all PE inputs bf16, PSUM/softmax f32):
  1. K^T = Wk^T @ xk^T   (channels on partitions)  -> bounce DRAM
     V' = [xv @ Wv | ones] (keys on partitions, per-head 65-col groups) -> bounce
  2. AllGather(K^T, V') within the 4-core batch group (single fused bf16 buffer)
  3. Q^T = Wq^T @ xq^T   (stays in SBUF; overlaps the collective)
  4. Per head pair: S^T = K @ Q^T (keys on partitions), exp on ScalarE
     (scale=1/8 folded in, no max-subtraction needed for N(0,1) scores),
     O'^T = V'^T @ P^T accumulated over key chunks; row 64 = softmax denom.
     Normalize with reciprocal broadcast, accumulate A^T in SBUF.
  5. out^T = Wo^T @ A^T -> DRAM (f32). Host transposes + concatenates.
"""

import sys

sys.path.insert(0, "/opt/trn_rl_repo")

from contextlib import ExitStack

import numpy as np
import ml_dtypes

import concourse.bass as bass
import concourse.bacc as bacc
import concourse.mybir as mybir
import concourse.tile as tile
from concourse.bass_utils import run_bass_kernel_spmd

BF16 = mybir.dt.bfloat16
F32 = mybir.dt.float32
Exp = mybir.ActivationFunctionType.Exp

B, N, C = 2, 2048, 1024
H, D = 16, 64
DV = D + 1          # V columns per head incl. ones column
NQ = N // 4         # queries per core = 512
NK = N              # keys per core (after gather)
NCHUNK = NK // 128  # 16 key chunks
KSZ = C * NQ        # K^T flat elems per rank = 524288
VSZ = N // 4 * (H * DV)  # V' flat elems per rank = 512*1040 = 532480
RSZ = KSZ + VSZ
SCALE = 1.0 / np.sqrt(D)

_CACHE = {}


def build_nc():
    nc = bacc.Bacc("TRN2", target_bir_lowering=False, debug=False, num_devices=8)

    xqT = nc.declare_dram_parameter("xqT", [C, NQ], BF16, isOutput=False)
    xkT = nc.declare_dram_parameter("xkT", [C, NQ], BF16, isOutput=False)
    xvT = nc.declare_dram_parameter("xvT", [C, NQ], BF16, isOutput=False)
    wq = nc.declare_dram_parameter("wq", [C, C], BF16, isOutput=False)
    wk = nc.declare_dram_parameter("wk", [C, C], BF16, isOutput=False)
    wv = nc.declare_dram_parameter("wv", [C, C], BF16, isOutput=False)
    wo = nc.declare_dram_parameter("wo", [C, C], BF16, isOutput=False)
    outT = nc.declare_dram_parameter("outT", [C, NQ], F32, isOutput=True)

    kv_bounce = nc.dram_tensor("kv_bounce", [RSZ], BF16)
    kv_gath = nc.dram_tensor("kv_gath", [4 * RSZ], BF16)

    # DRAM views
    kb_K = kv_bounce[:KSZ].rearrange("(c q) -> c q", q=NQ)          # (1024, 512)
    kb_V = kv_bounce[KSZ:].rearrange("(k v) -> k v", v=H * DV)      # (512, 1040)
    # gathered: rank-major flat blocks
    g2 = kv_gath[:].rearrange("(r f) -> r f", f=RSZ)                # (4, RSZ)

    with tile.TileContext(nc) as tc, ExitStack() as top:
        # ---------------- resident SBUF ----------------
        res = top.enter_context(tc.tile_pool(name="res", bufs=1))
        xq_sb = res.tile([128, 8 * NQ], BF16, tag="xq")   # x^T tiles: [:,512j] = rows 128j..
        xk_sb = res.tile([128, 8 * NQ], BF16, tag="xk")
        xv_sb = res.tile([128, 8 * NQ], BF16, tag="xv")
        qT_sb = res.tile([128, 8 * NQ], BF16, tag="qT")   # Q^T: tile i cols 512i
        aT_sb = res.tile([128, 8 * NQ], BF16, tag="aT")   # A^T accum
        dinv_sb = res.tile([64, NQ], F32, tag="dinv")
        drow_sb = res.tile([1, NQ], F32, tag="drow")

        for t, src in ((xk_sb, xkT), (xv_sb, xvT), (xq_sb, xqT)):
            for j in range(8):
                nc.sync.dma_start(out=t[:, NQ * j:NQ * (j + 1)],
                                  in_=src[128 * j:128 * (j + 1), :])

        # ---------------- projections K^T, V' ----------------
        with ExitStack() as ph:
            wpool = ph.enter_context(tc.tile_pool(name="wpool", bufs=10))
            ppool = ph.enter_context(tc.tile_pool(name="ppool", bufs=4, space="PSUM"))
            epool = ph.enter_context(tc.tile_pool(name="epool", bufs=4))

            wk_t = [wpool.tile([128, C], BF16, tag="w", name=f"wk_t{_i}") for _i in range(8)]
            for cc in range(8):
                nc.sync.dma_start(out=wk_t[cc][:], in_=wk[128 * cc:128 * (cc + 1), :])

            # K^T tile m: (128 ch, 512 q) = sum_cc Wk[cc,m]^T @ xk^T[cc]
            for m in range(8):
                ps = ppool.tile([128, NQ], F32, tag="ps")
                for cc in range(8):
                    nc.tensor.matmul(ps[:], wk_t[cc][:, 128 * m:128 * (m + 1)],
                                     xk_sb[:, NQ * cc:NQ * (cc + 1)],
                                     start=(cc == 0), stop=(cc == 7))
                ev = epool.tile([128, NQ], BF16, tag="ev")
                nc.vector.tensor_copy(ev[:], ps[:])
                nc.sync.dma_start(out=kb_K[128 * m:128 * (m + 1), :], in_=ev[:])

            wv_t = [wpool.tile([128, C], BF16, tag="w", name=f"wv_t{_i}") for _i in range(8)]
            for cc in range(8):
                nc.sync.dma_start(out=wv_t[cc][:], in_=wv[128 * cc:128 * (cc + 1), :])

            # V' tile kt: (128 keys, 1040) with ones cols; halves of 512 ch
            for kt in range(4):
                vsb = epool.tile([128, H * DV], BF16, tag="vsb")
                v3 = vsb[:].rearrange("p (h x) -> p h x", x=DV)
                nc.vector.memset(v3[:, :, D:DV], 1.0)
                for half in range(2):
                    ps = ppool.tile([128, NQ], F32, tag="ps")
                    for cc in range(8):
                        nc.tensor.matmul(
                            ps[:],
                            xv_sb[:, NQ * cc + 128 * kt:NQ * cc + 128 * (kt + 1)],
                            wv_t[cc][:, 512 * half:512 * (half + 1)],
                            start=(cc == 0), stop=(cc == 7))
                    nc.vector.tensor_copy(
                        v3[:, 8 * half:8 * (half + 1), 0:D],
                        ps[:].rearrange("p (h d) -> p h d", d=D))
                nc.sync.dma_start(out=kb_V[128 * kt:128 * (kt + 1), :], in_=vsb[:])

            # ---------------- collective ----------------
            nc.gpsimd.collective_compute(
                "AllGather", mybir.AluOpType.bypass,
                replica_groups=[[0, 1, 2, 3], [4, 5, 6, 7]],
                ins=[kv_bounce[:].opt()],
                outs=[kv_gath[:].opt()],
            )

            # ---------------- Q^T projection (overlaps collective) ----------------
            wq_t = [wpool.tile([128, C], BF16, tag="w", name=f"wq_t{_i}") for _i in range(8)]
            for cc in range(8):
                nc.sync.dma_start(out=wq_t[cc][:], in_=wq[128 * cc:128 * (cc + 1), :])
            for m in range(8):
                ps = ppool.tile([128, NQ], F32, tag="ps")
                for cc in range(8):
                    nc.tensor.matmul(ps[:], wq_t[cc][:, 128 * m:128 * (m + 1)],
                                     xq_sb[:, NQ * cc:NQ * (cc + 1)],
                                     start=(cc == 0), stop=(cc == 7))
                nc.vector.tensor_copy(qT_sb[:, NQ * m:NQ * (m + 1)], ps[:])

        # ---------------- attention, per head pair ----------------
        with ExitStack() as ph:
            kpool = ph.enter_context(tc.tile_pool(name="kpool", bufs=2))
            vpool = ph.enter_context(tc.tile_pool(name="vpool", bufs=2))
            spool = ph.enter_context(tc.tile_pool(name="spool", bufs=3, space="PSUM"))
            opool = ph.enter_context(tc.tile_pool(name="opool", bufs=2, space="PSUM"))
            P_pool = ph.enter_context(tc.tile_pool(name="P_pool", bufs=2))

            for i in range(8):  # head pair i = heads (2i, 2i+1)
                # K^T pair: (128 ch, 4 ranks, 512 keys)
                kp = kpool.tile([128, 4, NQ], BF16, tag="kp")
                src = g2[:, :KSZ].rearrange("r (c q) -> c r q", q=NQ)
                nc.sync.dma_start(out=kp[:], in_=src[128 * i:128 * (i + 1), :, :])
                # V' pair: (128 keys%128, 4 ranks, 4 chunks, 130)
                vp = vpool.tile([128, 4, 4, 2 * DV], BF16, tag="vp")
                vsrc = g2[:, KSZ:].rearrange("r (kc p v) -> p r kc v", p=128, v=H * DV)
                for r_ in range(4):
                    nc.sync.dma_start(
                        out=vp[:, r_],
                        in_=vsrc[:, r_, :, 2 * DV * i:2 * DV * (i + 1)])

                Pp = P_pool.tile([128, NCHUNK * 1024], BF16, tag="P")

                # S^T: psum tile kc covers heads (A, B) for one key chunk
                for kc in range(NCHUNK):
                    st = spool.tile([128, 1024], F32, tag="st")
                    r, c_ = kc // 4, kc % 4
                    key_sl = kp[:, r, 128 * c_:128 * (c_ + 1)]
                    nc.tensor.matmul(st[:, 0:512],
                                     key_sl[0:64, :],
                                     qT_sb[0:64, NQ * i:NQ * (i + 1)],
                                     start=True, stop=True)
                    nc.tensor.matmul(st[:, 512:1024],
                                     key_sl[64:128, :],
                                     qT_sb[64:128, NQ * i:NQ * (i + 1)],
                                     start=True, stop=True)
                    nc.scalar.activation(Pp[:, 1024 * kc:1024 * (kc + 1)], st[:],
                                         Exp, scale=float(SCALE))

                # PV: O'^T (65, 512) per head, accumulated over 16 chunks
                for h in range(2):
                    po = opool.tile([128, NQ], F32, tag="po")
                    for kc in range(NCHUNK):
                        r, c_ = kc // 4, kc % 4
                        nc.tensor.matmul(
                            po[0:DV, :],
                            vp[:, r, c_, DV * h:DV * (h + 1)],
                            Pp[:, 1024 * kc + 512 * h:1024 * kc + 512 * h + 512],
                            start=(kc == 0), stop=(kc == NCHUNK - 1))
                    nc.vector.reciprocal(drow_sb[:], po[D:DV, :])
                    nc.gpsimd.partition_broadcast(dinv_sb[:], drow_sb[:])
                    nc.vector.tensor_mul(
                        aT_sb[64 * h:64 * (h + 1), NQ * i:NQ * (i + 1)],
                        po[0:D, :], dinv_sb[:])

        # ---------------- output projection ----------------
        with ExitStack() as ph:
            wpool = ph.enter_context(tc.tile_pool(name="wopool", bufs=9))
            ppool = ph.enter_context(tc.tile_pool(name="popool", bufs=3, space="PSUM"))
            epool = ph.enter_context(tc.tile_pool(name="eopool", bufs=3))
            wo_t = [wpool.tile([128, C], BF16, tag="wo", name=f"wo_t{_i}") for _i in range(8)]
            for cc in range(8):
                nc.sync.dma_start(out=wo_t[cc][:], in_=wo[128 * cc:128 * (cc + 1), :])
            for m in range(8):
                ps = ppool.tile([128, NQ], F32, tag="ps")
                for cc in range(8):
                    nc.tensor.matmul(ps[:], wo_t[cc][:, 128 * m:128 * (m + 1)],
                                     aT_sb[:, NQ * cc:NQ * (cc + 1)],
                                     start=(cc == 0), stop=(cc == 7))
                ev = epool.tile([128, NQ], F32, tag="ev")
                nc.vector.tensor_copy(ev[:], ps[:])
                nc.sync.dma_start(out=outT[128 * m:128 * (m + 1), :], in_=ev[:])

    nc.compile()
    return nc


def _get_nc():
    if "nc" not in _CACHE:
        _CACHE["nc"] = build_nc()
    return _CACHE["nc"]


def _make_in_maps(q, k, v, Wq, Wk, Wv, Wo):
    bf = ml_dtypes.bfloat16
    wq_b = np.ascontiguousarray(Wq).astype(bf)
    wk_b = np.ascontiguousarray(Wk).astype(bf)
    wv_b = np.ascontiguousarray(Wv).astype(bf)
    wo_b = np.ascontiguousarray(Wo).astype(bf)
    in_maps = []
    for c in range(8):
        b, r = c // 4, c % 4
        sl = slice(NQ * r, NQ * (r + 1))
        in_maps.append({
            "xqT": np.ascontiguousarray(np.asarray(q)[b, sl, :].T).astype(bf),
            "xkT": np.ascontiguousarray(np.asarray(k)[b, sl, :].T).astype(bf),
            "xvT": np.ascontiguousarray(np.asarray(v)[b, sl, :].T).astype(bf),
            "wq": wq_b, "wk": wk_b, "wv": wv_b, "wo": wo_b,
        })
    return in_maps


def _run(inputs, trace=False, **kw):
    nc = _get_nc()
    in_maps = _make_in_maps(inputs["q"], inputs["k"], inputs["v"],
                            inputs["Wq"], inputs["Wk"], inputs["Wv"], inputs["Wo"])
    res = run_bass_kernel_spmd(nc, in_maps, core_ids=list(range(8)), trace=trace, **kw)
    out = np.empty((B, N, C), np.float32)
    for c in range(8):
        b, r = c // 4, c % 4
        out[b, NQ * r:NQ * (r + 1), :] = res.results[c]["outT"].T
    return out, res


def kernel(**inputs) -> np.ndarray:
    out, _ = _run(inputs, trace=False)
    return out


# revision 7
# speedup vs baseline: 1.0210x; 1.0210x over previous
"""Distributed multi-head attention kernel for 8 TRN2 NeuronCores.

Problem: B=2, N=2048, C=1024, H=16 heads, D=64.
  out = softmax((q@Wq)(k@Wk)^T / sqrt(D)) @ (v@Wv) @ Wo   (per head, biases are zero)

Sharding: sequence-parallel within batch (2 batch groups x 4 cores), with NO
collectives: measured AllGather on this fleet has a ~65us fixed cost plus slow
streaming (~110-145us total), while recomputing the full-batch K/V projections
locally costs ~109us of perfectly parallel PE time and keeps every core
independent.  Core c owns batch b=c//4, query rows R=[512r, 512r+512), r=c%4.

Per-core dataflow (all PE inputs bf16, PSUM/softmax f32):
  1. Q^T = Wq^T @ xq^T  (own rows; channels on partitions; stays in SBUF)
  2. K^T = Wk^T @ xk^T  for the FULL batch -> resident SBUF (128, 8*2048)
     S^T(pair i) = K @ Q^T interleaved right after K^T; exp on ScalarE
     (scale=1/8 folded in; no max-subtraction needed for ~N(0,1) scores)
  3. V' = [xv @ Wv | ones] full batch, per-head 65-col groups -> resident SBUF
  4. O'^T = V'^T @ P^T accumulated over key chunks; row 64 = softmax denom.
     Normalize with reciprocal broadcast; accumulate A^T in SBUF.
     PV(i) is software-pipelined against S^T(i+1) so the PE never waits on exp.
  5. out^T = Wo^T @ A^T -> DRAM (f32). Host transposes + concatenates.
"""

import sys

sys.path.insert(0, "/opt/trn_rl_repo")

from contextlib import ExitStack

import numpy as np
import ml_dtypes

import concourse.bass as bass
import concourse.bacc as bacc
import concourse.mybir as mybir
import concourse.tile as tile
from concourse.bass_utils import run_bass_kernel_spmd

BF16 = mybir.dt.bfloat16
F32 = mybir.dt.float32
Exp = mybir.ActivationFunctionType.Exp

B, N, C = 2, 2048, 1024
H, D = 16, 64
DV = D + 1          # V columns per head incl. ones column
NQ = N // 4         # queries per core = 512
NCHUNK = N // 128   # 16 key chunks
SCALE = 1.0 / np.sqrt(D)
PRELUDE = 1         # S^T pairs emitted before the V' projection phase

_CACHE = {}


def build_nc():
    nc = bacc.Bacc("TRN2", target_bir_lowering=False, debug=False, num_devices=8)

    xqT = nc.declare_dram_parameter("xqT", [C, NQ], BF16, isOutput=False)
    xkT = nc.declare_dram_parameter("xkT", [C, N], BF16, isOutput=False)
    xvT = nc.declare_dram_parameter("xvT", [C, N], BF16, isOutput=False)
    wq = nc.declare_dram_parameter("wq", [C, C], BF16, isOutput=False)
    wk = nc.declare_dram_parameter("wk", [C, C], BF16, isOutput=False)
    wv = nc.declare_dram_parameter("wv", [C, C], BF16, isOutput=False)
    wo = nc.declare_dram_parameter("wo", [C, C], BF16, isOutput=False)
    outT = nc.declare_dram_parameter("outT", [C, NQ], F32, isOutput=True)

    with tile.TileContext(nc) as tc, ExitStack() as top:
        # ---------------- resident SBUF (~83 KB/partition) ----------------
        res = top.enter_context(tc.tile_pool(name="res", bufs=1))
        qT_sb = res.tile([128, 8 * NQ], BF16, tag="qT")     # Q^T: pair i at cols 512i
        kT_sb = res.tile([128, 8 * N], BF16, tag="kT")      # K^T: pair i at cols 2048i
        v1_sb = res.tile([128, NCHUNK * H * DV], BF16, tag="v1")  # V' chunk kc at 1040*kc
        aT_sb = res.tile([128, 8 * NQ], BF16, tag="aT")     # A^T accum
        dinv_sb = res.tile([64, NQ], F32, tag="dinv")
        drow_sb = res.tile([1, NQ], F32, tag="drow")

        attn_stack = ExitStack()
        spool = attn_stack.enter_context(
            tc.tile_pool(name="spool", bufs=3, space="PSUM"))       # 6 banks
        P_pool = attn_stack.enter_context(
            tc.tile_pool(name="P_pool", bufs=PRELUDE + 1))          # 32 KB each

        def st_pair(i):
            """Emit S^T + exp for head pair i; returns the P tile."""
            Pp = P_pool.tile([128, NCHUNK * 1024], BF16, tag="P", name=f"P_{i}")
            for kc in range(NCHUNK):
                st = spool.tile([128, 1024], F32, tag="st", name=f"st_{i}_{kc}")
                key_sl = kT_sb[:, N * i + 128 * kc:N * i + 128 * (kc + 1)]
                nc.tensor.matmul(st[:, 0:512],
                                 key_sl[0:64, :],
                                 qT_sb[0:64, NQ * i:NQ * (i + 1)],
                                 start=True, stop=True)
                nc.tensor.matmul(st[:, 512:1024],
                                 key_sl[64:128, :],
                                 qT_sb[64:128, NQ * i:NQ * (i + 1)],
                                 start=True, stop=True)
                nc.scalar.activation(Pp[:, 1024 * kc:1024 * (kc + 1)], st[:],
                                     Exp, scale=float(SCALE))
            return Pp

        with ExitStack() as ph:
            wpool = ph.enter_context(tc.tile_pool(name="wpool", bufs=9))
            xqpool = ph.enter_context(tc.tile_pool(name="xqpool", bufs=8))
            xkpool = ph.enter_context(tc.tile_pool(name="xkpool", bufs=16))
            ppool = ph.enter_context(tc.tile_pool(name="ppool", bufs=2, space="PSUM"))

            # ---------------- Q^T projection ----------------
            xq_t = [xqpool.tile([128, NQ], BF16, tag="xq", name=f"xq_t{j}")
                    for j in range(8)]
            for j in range(8):
                nc.sync.dma_start(out=xq_t[j][:], in_=xqT[128 * j:128 * (j + 1), :])
            wq_t = [wpool.tile([128, C], BF16, tag="w", name=f"wq_t{j}")
                    for j in range(8)]
            for cc in range(8):
                nc.sync.dma_start(out=wq_t[cc][:], in_=wq[128 * cc:128 * (cc + 1), :])
            for m in range(8):
                ps = ppool.tile([128, NQ], F32, tag="ps", name=f"qps{m}")
                for cc in range(8):
                    nc.tensor.matmul(ps[:], wq_t[cc][:, 128 * m:128 * (m + 1)],
                                     xq_t[cc][:],
                                     start=(cc == 0), stop=(cc == 7))
                nc.vector.tensor_copy(qT_sb[:, NQ * m:NQ * (m + 1)], ps[:])

            # ------------- K^T projection (full batch) + early S^T -------------
            wk_t = [wpool.tile([128, C], BF16, tag="w", name=f"wk_t{j}")
                    for j in range(8)]
            for cc in range(8):
                nc.sync.dma_start(out=wk_t[cc][:], in_=wk[128 * cc:128 * (cc + 1), :])
            # x^T input staged in (128, 512) blocks: tile (j, qb)
            for qb in range(4):
                xk_t = [xkpool.tile([128, 512], BF16, tag="xk", name=f"xk{qb}_{j}")
                        for j in range(8)]
                for j in range(8):
                    nc.sync.dma_start(
                        out=xk_t[j][:],
                        in_=xkT[128 * j:128 * (j + 1), 512 * qb:512 * (qb + 1)])
                for m in range(8):
                    ps = ppool.tile([128, 512], F32, tag="ps", name=f"kps{m}_{qb}")
                    for cc in range(8):
                        nc.tensor.matmul(ps[:], wk_t[cc][:, 128 * m:128 * (m + 1)],
                                         xk_t[cc][:],
                                         start=(cc == 0), stop=(cc == 7))
                    nc.vector.tensor_copy(
                        kT_sb[:, N * m + 512 * qb:N * m + 512 * (qb + 1)], ps[:])

            P_tiles = {}
            for i in range(PRELUDE):
                P_tiles[i] = st_pair(i)

            # ---------------- V' projection (full batch) ----------------
            wv_t = [wpool.tile([128, C], BF16, tag="w", name=f"wv_t{j}")
                    for j in range(8)]
            for cc in range(8):
                nc.sync.dma_start(out=wv_t[cc][:], in_=wv[128 * cc:128 * (cc + 1), :])
            v3 = v1_sb[:].rearrange("p (kc h x) -> p kc h x", kc=NCHUNK, x=DV)
            nc.vector.memset(v3[:, :, :, D:DV], 1.0)
            for qb in range(4):
                xv_t = [xkpool.tile([128, 512], BF16, tag="xk", name=f"xv{qb}_{j}")
                        for j in range(8)]
                for j in range(8):
                    nc.sync.dma_start(
                        out=xv_t[j][:],
                        in_=xvT[128 * j:128 * (j + 1), 512 * qb:512 * (qb + 1)])
                for sub in range(4):
                    kc = 4 * qb + sub
                    for half in range(2):
                        ps = ppool.tile([128, 512], F32, tag="ps",
                                        name=f"vps{kc}_{half}")
                        for cc in range(8):
                            nc.tensor.matmul(
                                ps[:],
                                xv_t[cc][:, 128 * sub:128 * (sub + 1)],
                                wv_t[cc][:, 512 * half:512 * (half + 1)],
                                start=(cc == 0), stop=(cc == 7))
                        nc.vector.tensor_copy(
                            v3[:, kc, 8 * half:8 * (half + 1), 0:D],
                            ps[:].rearrange("p (h d) -> p h d", d=D))

        # ---------- attention: PV(i) pipelined against S^T(i+PRELUDE) ----------
        with ExitStack() as ph:
            opool = ph.enter_context(tc.tile_pool(name="opool", bufs=2, space="PSUM"))
            for i in range(8):
                Pp = P_tiles.pop(i)
                if i + PRELUDE < 8:
                    P_tiles[i + PRELUDE] = st_pair(i + PRELUDE)
                for h in range(2):
                    po = opool.tile([128, NQ], F32, tag="po", name=f"po{i}_{h}")
                    for kc in range(NCHUNK):
                        nc.tensor.matmul(
                            po[0:DV, :],
                            v1_sb[:].rearrange("p (kc v) -> p kc v", v=H * DV)
                                 [:, kc, 2 * DV * i + DV * h:2 * DV * i + DV * (h + 1)],
                            Pp[:, 1024 * kc + 512 * h:1024 * kc + 512 * h + 512],
                            start=(kc == 0), stop=(kc == NCHUNK - 1))
                    nc.vector.reciprocal(drow_sb[:], po[D:DV, :])
                    nc.gpsimd.partition_broadcast(dinv_sb[:], drow_sb[:])
                    nc.vector.tensor_mul(
                        aT_sb[64 * h:64 * (h + 1), NQ * i:NQ * (i + 1)],
                        po[0:D, :], dinv_sb[:])
        attn_stack.close()

        # ---------------- output projection ----------------
        with ExitStack() as ph:
            wpool = ph.enter_context(tc.tile_pool(name="wopool", bufs=9))
            ppool = ph.enter_context(tc.tile_pool(name="popool", bufs=3, space="PSUM"))
            epool = ph.enter_context(tc.tile_pool(name="eopool", bufs=3))
            wo_t = [wpool.tile([128, C], BF16, tag="wo", name=f"wo_t{j}")
                    for j in range(8)]
            for cc in range(8):
                nc.sync.dma_start(out=wo_t[cc][:], in_=wo[128 * cc:128 * (cc + 1), :])
            for m in range(8):
                ps = ppool.tile([128, NQ], F32, tag="ps", name=f"ops{m}")
                for cc in range(8):
                    nc.tensor.matmul(ps[:], wo_t[cc][:, 128 * m:128 * (m + 1)],
                                     aT_sb[:, NQ * cc:NQ * (cc + 1)],
                                     start=(cc == 0), stop=(cc == 7))
                ev = epool.tile([128, NQ], F32, tag="ev", name=f"oev{m}")
                nc.vector.tensor_copy(ev[:], ps[:])
                nc.sync.dma_start(out=outT[128 * m:128 * (m + 1), :], in_=ev[:])

    nc.compile()
    return nc


def _get_nc():
    if "nc" not in _CACHE:
        _CACHE["nc"] = build_nc()
    return _CACHE["nc"]


def _make_in_maps(q, k, v, Wq, Wk, Wv, Wo):
    bf = ml_dtypes.bfloat16
    wq_b = np.ascontiguousarray(Wq).astype(bf)
    wk_b = np.ascontiguousarray(Wk).astype(bf)
    wv_b = np.ascontiguousarray(Wv).astype(bf)
    wo_b = np.ascontiguousarray(Wo).astype(bf)
    q = np.asarray(q)
    kT = [np.ascontiguousarray(np.asarray(k)[b].T).astype(bf) for b in range(B)]
    vT = [np.ascontiguousarray(np.asarray(v)[b].T).astype(bf) for b in range(B)]
    in_maps = []
    for c in range(8):
        b, r = c // 4, c % 4
        sl = slice(NQ * r, NQ * (r + 1))
        in_maps.append({
            "xqT": np.ascontiguousarray(q[b, sl, :].T).astype(bf),
            "xkT": kT[b], "xvT": vT[b],
            "wq": wq_b, "wk": wk_b, "wv": wv_b, "wo": wo_b,
        })
    return in_maps


def _run(inputs, trace=False, **kw):
    nc = _get_nc()
    in_maps = _make_in_maps(inputs["q"], inputs["k"], inputs["v"],
                            inputs["Wq"], inputs["Wk"], inputs["Wv"], inputs["Wo"])
    res = run_bass_kernel_spmd(nc, in_maps, core_ids=list(range(8)), trace=trace, **kw)
    out = np.empty((B, N, C), np.float32)
    for c in range(8):
        b, r = c // 4, c % 4
        out[b, NQ * r:NQ * (r + 1), :] = res.results[c]["outT"].T
    return out, res


def kernel(**inputs) -> np.ndarray:
    out, _ = _run(inputs, trace=False)
    return out


# revision 10
# speedup vs baseline: 1.3412x; 1.3137x over previous
"""Distributed multi-head attention kernel for 8 TRN2 NeuronCores.

Problem: B=2, N=2048, C=1024, H=16 heads, D=64.
  out = softmax((q@Wq)(k@Wk)^T / sqrt(D)) @ (v@Wv) @ Wo   (per head, biases are zero)

Sharding: sequence-parallel within batch (2 batch groups x 4 cores), with NO
collectives: measured AllGather on this fleet has a ~65us fixed cost plus slow
streaming (~110-145us total), while recomputing the full-batch K/V projections
locally costs ~109us of perfectly parallel PE time and keeps every core
independent.  Core c owns batch b=c//4, query rows R=[512r, 512r+512), r=c%4.

Per-core dataflow (all PE inputs bf16, PSUM/softmax f32):
  1. Q^T = Wq^T @ xq^T  (own rows; channels on partitions; stays in SBUF)
  2. K^T = Wk^T @ xk^T  for the FULL batch -> resident SBUF (128, 8*2048)
     S^T(pair i) = K @ Q^T interleaved right after K^T; exp on ScalarE
     (scale=1/8 folded in; no max-subtraction needed for ~N(0,1) scores)
  3. V' = [xv @ Wv | ones] full batch, per-head 65-col groups -> resident SBUF
  4. O'^T = V'^T @ P^T accumulated over key chunks; row 64 = softmax denom.
     Normalize with reciprocal broadcast; accumulate A^T in SBUF.
     PV(i) is software-pipelined against S^T(i+1) so the PE never waits on exp.
  5. out^T = Wo^T @ A^T -> DRAM (f32). Host transposes + concatenates.
"""

import sys

sys.path.insert(0, "/opt/trn_rl_repo")

from contextlib import ExitStack

import numpy as np
import ml_dtypes

import concourse.bass as bass
import concourse.bacc as bacc
import concourse.mybir as mybir
import concourse.tile as tile
from concourse.bass_utils import run_bass_kernel_spmd

BF16 = mybir.dt.bfloat16
F32 = mybir.dt.float32
Exp = mybir.ActivationFunctionType.Exp

B, N, C = 2, 2048, 1024
H, D = 16, 64
DV = D + 1          # V columns per head incl. ones column
NQ = N // 4         # queries per core = 512
NCHUNK = N // 128   # 16 key chunks
SCALE = 1.0 / np.sqrt(D)
PRELUDE = 1         # S^T pairs emitted before the V' projection phase

_CACHE = {}


def build_nc():
    nc = bacc.Bacc("TRN2", target_bir_lowering=False, debug=False, num_devices=8)

    xqT = nc.declare_dram_parameter("xqT", [C, NQ], BF16, isOutput=False)
    xkT = nc.declare_dram_parameter("xkT", [C, N], BF16, isOutput=False)
    xvT = nc.declare_dram_parameter("xvT", [C, N], BF16, isOutput=False)
    wq = nc.declare_dram_parameter("wq", [C, C], BF16, isOutput=False)
    wk = nc.declare_dram_parameter("wk", [C, C], BF16, isOutput=False)
    wv = nc.declare_dram_parameter("wv", [C, C], BF16, isOutput=False)
    wo = nc.declare_dram_parameter("wo", [C, C], BF16, isOutput=False)
    outT = nc.declare_dram_parameter("outT", [C, NQ], F32, isOutput=True)

    with tile.TileContext(nc) as tc, ExitStack() as top:
        # ---------------- resident SBUF (~83 KB/partition) ----------------
        res = top.enter_context(tc.tile_pool(name="res", bufs=1))
        qT_sb = res.tile([128, 8 * NQ], BF16, tag="qT")     # Q^T: pair i at cols 512i
        kT_sb = res.tile([128, 8 * N], BF16, tag="kT")      # K^T: pair i at cols 2048i
        v1_sb = res.tile([128, NCHUNK * H * DV], BF16, tag="v1")  # V' chunk kc at 1040*kc
        aT_sb = res.tile([128, 8 * NQ], BF16, tag="aT")     # A^T accum
        dinv_sb = res.tile([64, NQ], F32, tag="dinv")
        drow_sb = res.tile([1, NQ], F32, tag="drow")
        draw_sb = res.tile([1, NQ], F32, tag="draw")

        attn_stack = ExitStack()
        spool = attn_stack.enter_context(
            tc.tile_pool(name="spool", bufs=3, space="PSUM"))       # 6 banks
        P_pool = attn_stack.enter_context(
            tc.tile_pool(name="P_pool", bufs=PRELUDE + 1))          # 32 KB each

        def st_pair(i):
            """Emit S^T + exp for head pair i; returns the P tile."""
            Pp = P_pool.tile([128, NCHUNK * 1024], BF16, tag="P", name=f"P_{i}")
            for kc in range(NCHUNK):
                st = spool.tile([128, 1024], F32, tag="st", name=f"st_{i}_{kc}")
                key_sl = kT_sb[:, N * i + 128 * kc:N * i + 128 * (kc + 1)]
                nc.tensor.matmul(st[:, 0:512],
                                 key_sl[0:64, :],
                                 qT_sb[0:64, NQ * i:NQ * (i + 1)],
                                 start=True, stop=True)
                nc.tensor.matmul(st[:, 512:1024],
                                 key_sl[64:128, :],
                                 qT_sb[64:128, NQ * i:NQ * (i + 1)],
                                 start=True, stop=True)
                nc.scalar.activation(Pp[:, 1024 * kc:1024 * (kc + 1)], st[:],
                                     Exp, scale=float(SCALE))
            return Pp

        with ExitStack() as ph:
            wpool = ph.enter_context(tc.tile_pool(name="wpool", bufs=9))
            xqpool = ph.enter_context(tc.tile_pool(name="xqpool", bufs=8))
            xkpool = ph.enter_context(tc.tile_pool(name="xkpool", bufs=16))
            ppool = ph.enter_context(tc.tile_pool(name="ppool", bufs=2, space="PSUM"))

            # ---------------- Q^T projection ----------------
            xq_t = [xqpool.tile([128, NQ], BF16, tag="xq", name=f"xq_t{j}")
                    for j in range(8)]
            for j in range(8):
                nc.sync.dma_start(out=xq_t[j][:], in_=xqT[128 * j:128 * (j + 1), :])
            wq_t = [wpool.tile([128, C], BF16, tag="w", name=f"wq_t{j}")
                    for j in range(8)]
            for cc in range(8):
                nc.sync.dma_start(out=wq_t[cc][:], in_=wq[128 * cc:128 * (cc + 1), :])
            for m in range(8):
                ps = ppool.tile([128, NQ], F32, tag="ps", name=f"qps{m}")
                for cc in range(8):
                    nc.tensor.matmul(ps[:], wq_t[cc][:, 128 * m:128 * (m + 1)],
                                     xq_t[cc][:],
                                     start=(cc == 0), stop=(cc == 7))
                nc.vector.tensor_copy(qT_sb[:, NQ * m:NQ * (m + 1)], ps[:])

            # ------------- K^T projection (full batch) + early S^T -------------
            wk_t = [wpool.tile([128, C], BF16, tag="w", name=f"wk_t{j}")
                    for j in range(8)]
            for cc in range(8):
                nc.sync.dma_start(out=wk_t[cc][:], in_=wk[128 * cc:128 * (cc + 1), :])
            # x^T input staged in (128, 512) blocks: tile (j, qb)
            for qb in range(4):
                xk_t = [xkpool.tile([128, 512], BF16, tag="xk", name=f"xk{qb}_{j}")
                        for j in range(8)]
                for j in range(8):
                    nc.sync.dma_start(
                        out=xk_t[j][:],
                        in_=xkT[128 * j:128 * (j + 1), 512 * qb:512 * (qb + 1)])
                for m in range(8):
                    ps = ppool.tile([128, 512], F32, tag="ps", name=f"kps{m}_{qb}")
                    for cc in range(8):
                        nc.tensor.matmul(ps[:], wk_t[cc][:, 128 * m:128 * (m + 1)],
                                         xk_t[cc][:],
                                         start=(cc == 0), stop=(cc == 7))
                    nc.vector.tensor_copy(
                        kT_sb[:, N * m + 512 * qb:N * m + 512 * (qb + 1)], ps[:])

            P_tiles = {}
            for i in range(PRELUDE):
                P_tiles[i] = st_pair(i)

            # ---------------- V' projection (full batch) ----------------
            wv_t = [wpool.tile([128, C], BF16, tag="w", name=f"wv_t{j}")
                    for j in range(8)]
            for cc in range(8):
                nc.sync.dma_start(out=wv_t[cc][:], in_=wv[128 * cc:128 * (cc + 1), :])
            v3 = v1_sb[:].rearrange("p (kc h x) -> p kc h x", kc=NCHUNK, x=DV)
            nc.vector.memset(v3[:, :, :, D:DV], 1.0)
            for qb in range(4):
                xv_t = [xkpool.tile([128, 512], BF16, tag="xk", name=f"xv{qb}_{j}")
                        for j in range(8)]
                for j in range(8):
                    nc.sync.dma_start(
                        out=xv_t[j][:],
                        in_=xvT[128 * j:128 * (j + 1), 512 * qb:512 * (qb + 1)])
                for sub in range(4):
                    kc = 4 * qb + sub
                    for half in range(2):
                        ps = ppool.tile([128, 512], F32, tag="ps",
                                        name=f"vps{kc}_{half}")
                        for cc in range(8):
                            nc.tensor.matmul(
                                ps[:],
                                xv_t[cc][:, 128 * sub:128 * (sub + 1)],
                                wv_t[cc][:, 512 * half:512 * (half + 1)],
                                start=(cc == 0), stop=(cc == 7))
                        nc.vector.tensor_copy(
                            v3[:, kc, 8 * half:8 * (half + 1), 0:D],
                            ps[:].rearrange("p (h d) -> p h d", d=D))

        # ---------- attention: PV(i) pipelined against S^T(i+PRELUDE) ----------
        with ExitStack() as ph:
            opool = ph.enter_context(tc.tile_pool(name="opool", bufs=2, space="PSUM"))
            for i in range(8):
                Pp = P_tiles.pop(i)
                if i + PRELUDE < 8:
                    P_tiles[i + PRELUDE] = st_pair(i + PRELUDE)
                for h in range(2):
                    po = opool.tile([128, NQ], F32, tag="po", name=f"po{i}_{h}")
                    for kc in range(NCHUNK):
                        nc.tensor.matmul(
                            po[0:DV, :],
                            v1_sb[:].rearrange("p (kc v) -> p kc v", v=H * DV)
                                 [:, kc, 2 * DV * i + DV * h:2 * DV * i + DV * (h + 1)],
                            Pp[:, 1024 * kc + 512 * h:1024 * kc + 512 * h + 512],
                            start=(kc == 0), stop=(kc == NCHUNK - 1))
                    nc.vector.tensor_copy(draw_sb[:], po[D:DV, :])
                    nc.vector.reciprocal_approx_fast(drow_sb[:], draw_sb[:])
                    nc.gpsimd.partition_broadcast(dinv_sb[:], drow_sb[:])
                    nc.vector.tensor_mul(
                        aT_sb[64 * h:64 * (h + 1), NQ * i:NQ * (i + 1)],
                        po[0:D, :], dinv_sb[:])
        attn_stack.close()

        # ---------------- output projection ----------------
        with ExitStack() as ph:
            wpool = ph.enter_context(tc.tile_pool(name="wopool", bufs=9))
            ppool = ph.enter_context(tc.tile_pool(name="popool", bufs=3, space="PSUM"))
            epool = ph.enter_context(tc.tile_pool(name="eopool", bufs=3))
            wo_t = [wpool.tile([128, C], BF16, tag="wo", name=f"wo_t{j}")
                    for j in range(8)]
            for cc in range(8):
                nc.sync.dma_start(out=wo_t[cc][:], in_=wo[128 * cc:128 * (cc + 1), :])
            for m in range(8):
                ps = ppool.tile([128, NQ], F32, tag="ps", name=f"ops{m}")
                for cc in range(8):
                    nc.tensor.matmul(ps[:], wo_t[cc][:, 128 * m:128 * (m + 1)],
                                     aT_sb[:, NQ * cc:NQ * (cc + 1)],
                                     start=(cc == 0), stop=(cc == 7))
                ev = epool.tile([128, NQ], F32, tag="ev", name=f"oev{m}")
                nc.vector.tensor_copy(ev[:], ps[:])
                nc.sync.dma_start(out=outT[128 * m:128 * (m + 1), :], in_=ev[:])

    nc.compile()
    return nc


def _get_nc():
    if "nc" not in _CACHE:
        _CACHE["nc"] = build_nc()
    return _CACHE["nc"]


def _make_in_maps(q, k, v, Wq, Wk, Wv, Wo):
    bf = ml_dtypes.bfloat16
    wq_b = np.ascontiguousarray(Wq).astype(bf)
    wk_b = np.ascontiguousarray(Wk).astype(bf)
    wv_b = np.ascontiguousarray(Wv).astype(bf)
    wo_b = np.ascontiguousarray(Wo).astype(bf)
    q = np.asarray(q)
    kT = [np.ascontiguousarray(np.asarray(k)[b].T).astype(bf) for b in range(B)]
    vT = [np.ascontiguousarray(np.asarray(v)[b].T).astype(bf) for b in range(B)]
    in_maps = []
    for c in range(8):
        b, r = c // 4, c % 4
        sl = slice(NQ * r, NQ * (r + 1))
        in_maps.append({
            "xqT": np.ascontiguousarray(q[b, sl, :].T).astype(bf),
            "xkT": kT[b], "xvT": vT[b],
            "wq": wq_b, "wk": wk_b, "wv": wv_b, "wo": wo_b,
        })
    return in_maps


def _run(inputs, trace=False, **kw):
    nc = _get_nc()
    in_maps = _make_in_maps(inputs["q"], inputs["k"], inputs["v"],
                            inputs["Wq"], inputs["Wk"], inputs["Wv"], inputs["Wo"])
    res = run_bass_kernel_spmd(nc, in_maps, core_ids=list(range(8)), trace=trace, **kw)
    out = np.empty((B, N, C), np.float32)
    for c in range(8):
        b, r = c // 4, c % 4
        out[b, NQ * r:NQ * (r + 1), :] = res.results[c]["outT"].T
    return out, res


def kernel(**inputs) -> np.ndarray:
    out, _ = _run(inputs, trace=False)
    return out
